# revision 39
# baseline (speedup 1.0000x reference)
"""Multi-head attention (dense transformer block) on 8 Trainium2 NeuronCores.

Reference computation (per batch element b of 8):
    qkv = x @ w_qkv.T + b_qkv                  # [1024, 2304]
    q, k, v = split heads (12 heads, d=64)
    attn = softmax(q k^T / sqrt(d))
    out  = (attn v) reshaped @ w_proj.T + b_proj

Sharding: pure data parallel — core b handles batch element b, weights are
replicated, no collectives.

Per-core kernel (all matmul operands fp16, fp32 PSUM accumulation):
  C: v    = x Wv^T + b_v            -> [1024, 12*(64+1)] (ones col per head
     makes the PV matmul emit softmax row-sums for free)
  B: qk^T = [Wq*scale; Wk] x^T      -> [1536, 1024] (features on partitions)
  D: per head pair hp: scores^T = k^T q (two heads row-tiled into the two
     halves of the PE array), exp on ScalarE straight from PSUM pairs,
     PV accumulate, normalize by approx-reciprocal(rowsum).
     B tiles for wave hp+1 and PV of wave hp-1 are interleaved between the
     score matmuls so the PE never idles while ScalarE runs the exps.
  E: out = score w_proj^T, evicted fp16 by ScalarE copies (b_proj is added
     host-side), DMA'd per o-half so the tail drain is short.

All inputs are repacked host-side into the exact SBUF layouts ([128
partitions, 6 contraction chunks, free]) so every DMA is a plain strided
copy, issued in consumption order; warmup matmuls on junk data run during
the initial DMA window to lift the PE clock gate (HAM) to 2.4 GHz before
real work arrives.
"""

import os
import sys

for _p in ("/opt/trn_rl_repo", "/root/.axon_site/_ro/trn_rl_repo"):
    if os.path.isdir(_p) and _p not in sys.path:
        sys.path.insert(0, _p)

import numpy as np

import concourse.bass as bass
import concourse.mybir as mybir
import concourse.tile as tile
from concourse import bacc
from concourse.bass_utils import run_bass_kernel_spmd

DIM = 768
N_HEAD = 12
HEAD_DIM = 64
SCALE = HEAD_DIM ** (-0.5)
NB = 8          # batch == number of cores
N = 1024        # sequence length
CCH = DIM // 128  # 6 contraction chunks

F32 = mybir.dt.float32
F16 = mybir.dt.float16
AF = mybir.ActivationFunctionType

# qk o-tiles (12 of 128) are laid out in SBUF in wave consumption order so
# the wqk DMA splits can land just in time.
OT_ORDER = [0, 6, 1, 7, 2, 8, 3, 9, 4, 10, 5, 11]
SLOT = {ot: s for s, ot in enumerate(OT_ORDER)}

_CACHE: dict = {}


def _build():
    nc = bacc.Bacc("TRN2", target_bir_lowering=False, debug=False)

    xp_d = nc.dram_tensor("xp", [128, 4, CCH, 256], F16, kind="ExternalInput")
    wqk_d = nc.dram_tensor("wqk_p", [128, CCH, 2 * DIM], F16, kind="ExternalInput")
    bqk_d = nc.dram_tensor("b_qk_t", [128, 12], F32, kind="ExternalInput")
    wv_d = nc.dram_tensor("wv_p", [128, CCH, DIM], F16, kind="ExternalInput")
    wp_d = nc.dram_tensor("wp_p", [128, CCH, DIM], F16, kind="ExternalInput")
    bvp_d = nc.dram_tensor("b_vp", [1, 2 * DIM], F16, kind="ExternalInput")
    out_d = nc.dram_tensor("out", [N, DIM], F16, kind="ExternalOutput")

    with tile.TileContext(nc) as tc:
        with (
            tc.tile_pool(name="consts", bufs=1) as consts,
            tc.tile_pool(name="qk", bufs=1) as qk_pool,
            tc.tile_pool(name="score", bufs=1) as score_pool,
            tc.tile_pool(name="v", bufs=1) as v_pool,
            tc.tile_pool(name="x", bufs=1) as x_pool,
            tc.tile_pool(name="wqk", bufs=1) as wqk_pool,
            tc.tile_pool(name="wv", bufs=1) as wv_pool,
            tc.tile_pool(name="wp", bufs=1) as wp_pool,
            tc.tile_pool(name="attn", bufs=32) as attn_pool,
            tc.tile_pool(name="small", bufs=4) as small_pool,
            tc.tile_pool(name="ostage", bufs=2) as out_pool,
            tc.tile_pool(name="ps", bufs=2, space="PSUM") as ps_pool,
            tc.tile_pool(name="pair", bufs=2, space="PSUM") as pair_pool,
            tc.tile_pool(name="acc", bufs=2, space="PSUM") as acc_pool,
        ):
            x_sb = x_pool.tile([128, 4, CCH, 256], F16)
            wv_sb = wv_pool.tile([128, CCH, DIM], F16)
            wqk_sb = wqk_pool.tile([128, CCH, 2 * DIM], F16)
            wp_sb = wp_pool.tile([128, CCH, DIM], F16)

            bqk_sb = consts.tile([128, 12], F32)
            bvp_sb = consts.tile([1, 2 * DIM], F16)

            # warmup junk operand: memset'd, costs ~100ns on the DVE
            warm_sb = consts.tile([128, 512], F16)
            nc.vector.memset(warm_sb[:], 0.0)

            # input DMAs in consumption order; each is a plain strided copy
            # of a host-prepacked [128, c, free] block
            nc.sync.dma_start(x_sb[:, 0], xp_d[:, 0])
            nc.sync.dma_start(wv_sb[:, :, 0:512], wv_d[:, :, 0:512])
            nc.sync.dma_start(bvp_sb[:], bvp_d[:])
            nc.sync.dma_start(bqk_sb[:], bqk_d[:])
            nc.sync.dma_start(x_sb[:, 1], xp_d[:, 1])
            nc.sync.dma_start(wqk_sb[:, :, 0:256], wqk_d[:, :, 0:256])
            nc.sync.dma_start(x_sb[:, 2:4], xp_d[:, 2:4])
            nc.sync.dma_start(wqk_sb[:, :, 256:512], wqk_d[:, :, 256:512])
            nc.sync.dma_start(wv_sb[:, :, 512:DIM], wv_d[:, :, 512:DIM])
            nc.sync.dma_start(wqk_sb[:, :, 512:1024], wqk_d[:, :, 512:1024])
            nc.sync.dma_start(wqk_sb[:, :, 1024:1536], wqk_d[:, :, 1024:1536])
            nc.sync.dma_start(wp_sb[:], wp_d[:])

            # lift the HAM clock gate while the first DMAs land; junk MMs
            # have no input semaphores so they run right after the preamble
            for _ in range(8):
                ps = ps_pool.tile([128, 512], F32)
                nc.tensor.matmul(
                    ps[:], warm_sb[:, 0:128], warm_sb[:], start=True, stop=True,
                )

            # broadcast b_v across partitions once; the C evictions add it
            # on the DVE
            bvp32 = consts.tile([1, 2 * DIM], F32)
            nc.vector.tensor_copy(bvp32[:], bvp_sb[:])
            bvb = consts.tile([128, DIM], F32)
            nc.gpsimd.partition_broadcast(bvb[:], bvp32[:, 0:DIM], channels=128)

            qk_sb = qk_pool.tile([128, 12, N], F16)         # [o=1536, n]
            score_sb = score_pool.tile([128, CCH, N], F16)  # [c=768, n]
            v_sb = v_pool.tile([128, 8, N_HEAD * 65], F16)  # [n, h*(64+1)]

            # ---- Phase C: v projection, natural layout + ones cols ----
            v_ones = v_sb[:].rearrange("p n (h d) -> p n h d", d=65)[:, :, :, 64:65]
            nc.vector.memset(v_ones, 1.0)

            def c_group(nt, blk):
                o0, ow, off = ((0, 512, 0), (512, 256, 8 * 65))[blk]
                ps = ps_pool.tile([128, 512], F32)
                n0 = (nt % 2) * 128
                for c in range(CCH):
                    nc.tensor.matmul(
                        ps[:, :ow],
                        x_sb[:, nt // 2, c, n0:n0 + 128],
                        wv_sb[:, c, o0:o0 + ow],
                        start=(c == 0),
                        stop=(c == CCH - 1),
                    )
                nh = ow // 64
                src = ps[:, :ow].rearrange("p (h d) -> p h d", d=64)
                bias = bvb[:, o0:o0 + ow].rearrange("p (h d) -> p h d", d=64)
                dst = v_sb[:, nt, off:off + nh * 65].rearrange(
                    "p (h d) -> p h d", d=65
                )[:, :, 0:64]
                nc.vector.tensor_add(dst, src, bias)

            # blk-0 groups for nt 0..3 run upfront (they only need the first
            # x/wv descriptors); everything else becomes wave-0/1 fillers
            for nt in range(4):
                c_group(nt, 0)

            # ---- Phase B helper: one [o-tile, nq] strip of the qk^T proj ----
            def b_group(ot, nq):
                sl = SLOT[ot]
                ps = ps_pool.tile([128, 512], F32)
                g0 = nq // 256
                for c in range(CCH):
                    nc.tensor.matmul(
                        ps[:],
                        wqk_sb[:, c, sl * 128:(sl + 1) * 128],
                        x_sb[:, g0:g0 + 2, c, :],
                        start=(c == 0),
                        stop=(c == CCH - 1),
                    )
                nc.vector.tensor_scalar_add(
                    qk_sb[:, ot, nq:nq + 512], ps[:], bqk_sb[:, ot:ot + 1],
                )

            # ---- Phase D helpers ----
            def score_pair(hp, nq, nk):
                """scoresT for both heads of pair hp, one nk tile: head A into
                cols 0:512 (PE rows 0-63), head B into 512:1024 (rows 64-127),
                then exp straight from the 2-bank PSUM pair into fp16 SBUF."""
                pair = pair_pool.tile([128, 1024], F32)
                for half, p0 in ((0, 0), (1, 64)):
                    nc.tensor.matmul(
                        pair[:, half * 512:(half + 1) * 512],
                        qk_sb[p0:p0 + 64, 6 + hp, nk * 128:(nk + 1) * 128],
                        qk_sb[p0:p0 + 64, hp, nq:nq + 512],
                        start=True, stop=True,
                        tile_position=(p0, 0),
                    )
                at = attn_pool.tile([128, 1024], F16)
                nc.scalar.activation(at[:], pair[:], AF.Exp)
                return at

            def pv_group(hp, nq, half, p0, attns, pool=None):
                """attn @ [v|1] for one head/nq strip + normalize by rowsum."""
                h = 2 * hp + half
                acc = (pool or acc_pool).tile([65, 512], F32, name="pair" if pool else "acc")
                for nk in range(8):
                    nc.tensor.matmul(
                        acc[:],
                        v_sb[:, nk, h * 65:(h + 1) * 65],
                        attns[nk][:, half * 512:(half + 1) * 512],
                        start=(nk == 0),
                        stop=(nk == 7),
                    )
                # custom-DVE ops mis-read PSUM APs at partition offsets > 0 —
                # stage the rowsum row to SBUF first.
                rs = small_pool.tile([1, 512], F32, tag="rs")
                nc.vector.tensor_copy(rs[:], acc[64:65, :])
                rec = small_pool.tile([1, 512], F32, tag="rec")
                nc.vector.reciprocal_approx_fast(rec[:], rs[:])
                bc = small_pool.tile([64, 512], F32, tag="bc")
                nc.gpsimd.partition_broadcast(bc[:], rec[:], channels=64)
                nc.vector.tensor_mul(
                    score_sb[p0:p0 + 64, hp, nq:nq + 512], acc[0:64, :], bc[:],
                )

            def num_pass(num, hp, nq, strips_nq, nk):
                """both heads' PV numerators full-width: A -> rows 0-63,
                B -> rows 64-127 via concurrent col tiles."""
                for half, c0 in ((0, 0), (1, 64)):
                    h = 2 * hp + half
                    nc.tensor.matmul(
                        num[c0:c0 + 64, :],
                        v_sb[:, nk, h * 65:h * 65 + 64],
                        strips_nq[nk][:, half * 512:(half + 1) * 512],
                        start=(nk == 0), stop=(nk == 7),
                        tile_position=(0, c0),
                    )

            def d_pass(dacc, hp, strips, nk):
                """softmax rowsums for both heads x both nq halves in one
                bank: 4 single-col tiles, each accumulating all 8 chunks."""
                for g, (nq, half) in enumerate(
                    ((0, 0), (0, 1), (512, 0), (512, 1))
                ):
                    h = 2 * hp + half
                    nc.tensor.matmul(
                        dacc[32 * g:32 * g + 1, :],
                        v_sb[:, nk, h * 65 + 64:h * 65 + 65],
                        strips[nq][nk][:, half * 512:(half + 1) * 512],
                        start=(nk == 0), stop=(nk == 7),
                        tile_position=(0, 32 * g),
                    )

            def d_chain(dacc, g):
                rs = small_pool.tile([1, 512], F32, tag="rs")
                nc.vector.tensor_copy(rs[:], dacc[32 * g:32 * g + 1, :])
                rec = small_pool.tile([1, 512], F32, tag="rec")
                nc.vector.reciprocal_approx_fast(rec[:], rs[:])
                bc = small_pool.tile([64, 512], F32, tag="bc")
                nc.gpsimd.partition_broadcast(bc[:], rec[:], channels=64)
                return bc

            def num_muls(num, hp, nq, bcs):
                for half, c0 in ((0, 0), (1, 64)):
                    nc.vector.tensor_mul(
                        score_sb[c0:c0 + 64, hp, nq:nq + 512],
                        num[c0:c0 + 64, :], bcs[half][:],
                    )

            # ---- Phases B + D interleaved in waves over head pairs ----
            # wave 0 starts as soon as b(0,0)/b(6,0) land: pairs that need
            # only the first x/wqk descriptors run first, so the ScalarE exp
            # stream (the wave-region bottleneck) starts ~7us earlier
            b_group(0, 0)
            b_group(6, 0)
            w0_pairs = (
                [(0, nk) for nk in range(4)]
                + [(512, nk) for nk in range(4)]
                + [(0, nk) for nk in range(4, 8)]
                + [(512, nk) for nk in range(4, 8)]
            )
            w0_fillers = [lambda: b_group(0, 512), lambda: b_group(6, 512)]
            for ot in (1, 7):
                for nq in (0, 512):
                    w0_fillers.append(lambda ot=ot, nq=nq: b_group(ot, nq))
            for nt in (0, 1, 2, 3):
                w0_fillers.append(lambda nt=nt: c_group(nt, 1))
            for nt in (4, 5, 6, 7):
                w0_fillers.append(lambda nt=nt: c_group(nt, 0))
            for nt in (4, 5, 6, 7):
                w0_fillers.append(lambda nt=nt: c_group(nt, 1))
            strips = {0: [None] * 8, 512: [None] * 8}
            fi = 0
            for si, (nq, nk) in enumerate(w0_pairs):
                strips[nq][nk] = score_pair(0, nq, nk)
                if fi < len(w0_fillers):
                    w0_fillers[fi]()
                    fi += 1
            while fi < len(w0_fillers):
                w0_fillers[fi]()
                fi += 1
            prev_strips = (0, strips)
            for hp in range(1, 5):
                # fillers keep the PE busy while ScalarE exps this wave
                fillers = []
                bfill = []
                if hp < 5:
                    for ot in (hp + 1, 7 + hp):
                        for nq in (0, 512):
                            bfill.append(lambda ot=ot, nq=nq: b_group(ot, nq))
                pfill = []
                if prev_strips is not None:
                    php, pstrips = prev_strips
                    # D passes first: the D bank evicts mid-wave so the two
                    # acc slots cover D, num(nq0), num(nq512) in rotation
                    dacc = acc_pool.tile([128, 512], F32, name="acc")
                    bcs = {}
                    for nk in range(8):
                        pfill.append(
                            lambda nk=nk, d=dacc, php=php, s=pstrips:
                                d_pass(d, php, s, nk))

                    def d_evict(d=dacc):
                        for g, key in enumerate(((0, 0), (0, 1), (512, 0), (512, 1))):
                            bcs[key] = d_chain(d, g)
                    pfill.append(d_evict)
                    num0 = acc_pool.tile([128, 512], F32, name="acc")
                    for nk in range(8):
                        pfill.append(
                            lambda nk=nk, n=num0, php=php, s=pstrips:
                                num_pass(n, php, 0, s[0], nk))
                    pfill.append(lambda n=num0, php=php: num_muls(
                        n, php, 0, (bcs[(0, 0)], bcs[(0, 1)])))
                    num1 = acc_pool.tile([128, 512], F32, name="acc")
                    for nk in range(8):
                        pfill.append(
                            lambda nk=nk, n=num1, php=php, s=pstrips:
                                num_pass(n, php, 512, s[512], nk))
                    pfill.append(lambda n=num1, php=php: num_muls(
                        n, php, 512, (bcs[(512, 0)], bcs[(512, 1)])))
                # spread the B groups evenly among the fine-grained PV units
                merged = list(pfill)
                step = max(1, (len(merged) + 1) // (len(bfill) + 1)) if bfill else 1
                for i, b in enumerate(bfill):
                    merged.insert(min(len(merged), (i + 1) * step + i), b)
                fillers.extend(merged)
                strips = {0: [], 512: []}
                fi = 0
                for si, (nq, nk) in enumerate(
                    [(nq, nk) for nq in (0, 512) for nk in range(8)]
                ):
                    strips[nq].append(score_pair(hp, nq, nk))
                    for _ in range(2):
                        if fi < len(fillers):
                            fillers[fi]()
                            fi += 1
                while fi < len(fillers):
                    fillers[fi]()
                    fi += 1
                prev_strips = (hp, strips)

            # ---- Phase E helper: ScalarE copy eviction (bias added on the
            # host), fp16 staging, DMA per o-half ----
            def e_tile(nt):
                stage = out_pool.tile([128, DIM], F16)
                for o0, ow in ((0, 512), (512, 256)):
                    ps = ps_pool.tile([128, 512], F32)
                    for c in range(CCH):
                        nc.tensor.matmul(
                            ps[:, :ow],
                            score_sb[:, c, nt * 128:(nt + 1) * 128],
                            wp_sb[:, c, o0:o0 + ow],
                            start=(c == 0),
                            stop=(c == CCH - 1),
                        )
                    nc.scalar.activation(
                        stage[:, o0:o0 + ow], ps[:, :ow], AF.Copy,
                    )
                    nc.sync.dma_start(
                        out_d[nt * 128:(nt + 1) * 128, o0:o0 + ow],
                        stage[:, o0:o0 + ow],
                    )

            # ---- wave 5: scores for hp=5; pv(4) fills the nq0 half, then
            # pv(5,nq0) fills the nq1 half; the tail interleaves pv(5,nq1)
            # with E tiles (whose nq halves of score_sb are already final) ----
            _, p4 = prev_strips
            strips5 = {0: [], 512: []}
            for nk in range(8):
                strips5[0].append(score_pair(5, 0, nk))
                if nk == 3:
                    pv_group(4, 0, 0, 0, p4[0])
                elif nk == 7:
                    pv_group(4, 0, 1, 64, p4[0])
            for nk in range(8):
                strips5[512].append(score_pair(5, 512, nk))
                if nk == 3:
                    pv_group(5, 0, 0, 0, strips5[0])
                elif nk == 7:
                    pv_group(5, 0, 1, 64, strips5[0])
            # pv4(nq1) waits until here: its strips are still alive and only
            # E tiles nt>=4 consume its output, so its chains hide under E
            pv_group(5, 512, 0, 0, strips5[512], pool=pair_pool)
            e_tile(0)
            pv_group(4, 512, 0, 0, p4[512])
            pv_group(5, 512, 1, 64, strips5[512], pool=pair_pool)
            e_tile(1)
            pv_group(4, 512, 1, 64, p4[512])
            for nt in (2, 3, 4, 5, 6, 7):
                e_tile(nt)

    nc.compile()
    return nc


def _get_nc():
    if "nc" not in _CACHE:
        _CACHE["nc"] = _build()
    return _CACHE["nc"]


def _pack6(w):
    """[768, o] -> [128, 6, o] with pack[p, c, o] = w[128c+p, o]."""
    return np.ascontiguousarray(w.reshape(CCH, 128, -1).transpose(1, 0, 2))


def kernel(x, w_qkv, b_qkv, w_proj, b_proj, **run_kwargs):
    x = np.asarray(x, dtype=np.float32)
    w_qkv = np.asarray(w_qkv, dtype=np.float32)
    b_qkv = np.asarray(b_qkv, dtype=np.float32)
    w_proj = np.asarray(w_proj, dtype=np.float32)
    b_proj = np.asarray(b_proj, dtype=np.float32)

    # Host-side layout prep (no arithmetic beyond folding the 1/sqrt(d) scale
    # into the q projection).
    w_qk = w_qkv[: 2 * DIM].copy()
    b_qk = b_qkv[: 2 * DIM].copy()
    w_qk[:DIM] *= SCALE
    b_qk[:DIM] *= SCALE
    # wqk in wave-order o-slots: pack[p, c, 128*slot+j] = w_qk[128*ot+j, 128c+p]
    wq4 = w_qk.reshape(12, 128, DIM)[OT_ORDER]            # [slot, j, c-dim]
    wqk_p = np.ascontiguousarray(
        wq4.transpose(2, 0, 1).reshape(DIM, 2 * DIM)      # [c-dim, slot*128+j]
    )
    wqk_p = _pack6(wqk_p).astype(np.float16)              # [128, 6, 1536]
    b_qk_t = np.ascontiguousarray(b_qk.reshape(12, 128).T)  # [128, 12] f32
    wv_p = _pack6(np.ascontiguousarray(w_qkv[2 * DIM:].T)).astype(np.float16)
    wp_p = _pack6(np.ascontiguousarray(w_proj.T)).astype(np.float16)
    b_vp = np.concatenate([b_qkv[2 * DIM:], b_proj]).reshape(1, -1).astype(np.float16)

    nc = _get_nc()
    in_maps = []
    for b in range(NB):
        xp = _pack6(np.ascontiguousarray(x[b].T)).astype(np.float16)
        # [128, 6, 1024] -> [128, 4 n-groups, 6, 256] for 3KB DMA runs
        xp = np.ascontiguousarray(
            xp.reshape(128, CCH, 4, 256).transpose(0, 2, 1, 3))
        in_maps.append({
            "xp": xp,
            "wqk_p": wqk_p,
            "b_qk_t": b_qk_t,
            "wv_p": wv_p,
            "wp_p": wp_p,
            "b_vp": b_vp,
        })
    res = run_bass_kernel_spmd(nc, in_maps, core_ids=list(range(NB)), **run_kwargs)
    out = np.stack(
        [res.results[b]["out"].astype(np.float32) for b in range(NB)], axis=0
    )
    out += b_proj
    if run_kwargs:
        return out, res
    return out


if __name__ == "__main__":
    rng = np.random.default_rng(0)
    x = rng.standard_normal((NB, N, DIM), dtype=np.float32)
    w_qkv = rng.standard_normal((3 * DIM, DIM), dtype=np.float32) * DIM ** -0.5
    b_qkv = rng.standard_normal((3 * DIM,), dtype=np.float32) * 0.02
    w_proj = rng.standard_normal((DIM, DIM), dtype=np.float32) * DIM ** -0.5
    b_proj = rng.standard_normal((DIM,), dtype=np.float32) * 0.02
    out = kernel(x=x, w_qkv=w_qkv, b_qkv=b_qkv, w_proj=w_proj, b_proj=b_proj)
    print("out", out.shape, out.dtype, float(np.abs(out).mean()))


# revision 40
# speedup vs baseline: 1.0122x; 1.0122x over previous
"""Multi-head attention (dense transformer block) on 8 Trainium2 NeuronCores.

Reference computation (per batch element b of 8):
    qkv = x @ w_qkv.T + b_qkv                  # [1024, 2304]
    q, k, v = split heads (12 heads, d=64)
    attn = softmax(q k^T / sqrt(d))
    out  = (attn v) reshaped @ w_proj.T + b_proj

Sharding: pure data parallel — core b handles batch element b, weights are
replicated, no collectives.

Per-core kernel (all matmul operands fp16, fp32 PSUM accumulation):
  C: v    = x Wv^T + b_v            -> [1024, 12*(64+1)] (ones col per head
     makes the PV matmul emit softmax row-sums for free)
  B: qk^T = [Wq*scale; Wk] x^T      -> [1536, 1024] (features on partitions)
  D: per head pair hp: scores^T = k^T q (two heads row-tiled into the two
     halves of the PE array), exp on ScalarE straight from PSUM pairs,
     PV accumulate, normalize by approx-reciprocal(rowsum).
     B tiles for wave hp+1 and PV of wave hp-1 are interleaved between the
     score matmuls so the PE never idles while ScalarE runs the exps.
  E: out = score w_proj^T, evicted fp16 by ScalarE copies (b_proj is added
     host-side), DMA'd per o-half so the tail drain is short.

All inputs are repacked host-side into the exact SBUF layouts ([128
partitions, 6 contraction chunks, free]) so every DMA is a plain strided
copy, issued in consumption order; warmup matmuls on junk data run during
the initial DMA window to lift the PE clock gate (HAM) to 2.4 GHz before
real work arrives.
"""

import os
import sys

for _p in ("/opt/trn_rl_repo", "/root/.axon_site/_ro/trn_rl_repo"):
    if os.path.isdir(_p) and _p not in sys.path:
        sys.path.insert(0, _p)

import numpy as np

import concourse.bass as bass
import concourse.mybir as mybir
import concourse.tile as tile
from concourse import bacc
from concourse.bass_utils import run_bass_kernel_spmd

DIM = 768
N_HEAD = 12
HEAD_DIM = 64
SCALE = HEAD_DIM ** (-0.5)
NB = 8          # batch == number of cores
N = 1024        # sequence length
CCH = DIM // 128  # 6 contraction chunks

F32 = mybir.dt.float32
F16 = mybir.dt.float16
AF = mybir.ActivationFunctionType

# qk o-tiles (12 of 128) are laid out in SBUF in wave consumption order so
# the wqk DMA splits can land just in time.
OT_ORDER = [0, 6, 1, 7, 2, 8, 3, 9, 4, 10, 5, 11]
SLOT = {ot: s for s, ot in enumerate(OT_ORDER)}

_CACHE: dict = {}


def _build():
    nc = bacc.Bacc("TRN2", target_bir_lowering=False, debug=False)

    xp_d = nc.dram_tensor("xp", [128, 4, CCH, 256], F16, kind="ExternalInput")
    wqk_d = nc.dram_tensor("wqk_p", [128, CCH, 2 * DIM], F16, kind="ExternalInput")
    bqk_d = nc.dram_tensor("b_qk_t", [128, 12], F32, kind="ExternalInput")
    wv_d = nc.dram_tensor("wv_p", [128, CCH, DIM], F16, kind="ExternalInput")
    wp_d = nc.dram_tensor("wp_p", [128, CCH, DIM], F16, kind="ExternalInput")
    bvp_d = nc.dram_tensor("b_vp", [1, 2 * DIM], F16, kind="ExternalInput")
    out_d = nc.dram_tensor("out", [N, DIM], F16, kind="ExternalOutput")

    with tile.TileContext(nc) as tc:
        with (
            tc.tile_pool(name="consts", bufs=1) as consts,
            tc.tile_pool(name="qk", bufs=1) as qk_pool,
            tc.tile_pool(name="score", bufs=1) as score_pool,
            tc.tile_pool(name="v", bufs=1) as v_pool,
            tc.tile_pool(name="x", bufs=1) as x_pool,
            tc.tile_pool(name="wqk", bufs=1) as wqk_pool,
            tc.tile_pool(name="wv", bufs=1) as wv_pool,
            tc.tile_pool(name="wp", bufs=1) as wp_pool,
            tc.tile_pool(name="attn", bufs=32) as attn_pool,
            tc.tile_pool(name="small", bufs=4) as small_pool,
            tc.tile_pool(name="ostage", bufs=2) as out_pool,
            tc.tile_pool(name="ps", bufs=2, space="PSUM") as ps_pool,
            tc.tile_pool(name="pair", bufs=2, space="PSUM") as pair_pool,
            tc.tile_pool(name="acc", bufs=2, space="PSUM") as acc_pool,
        ):
            x_sb = x_pool.tile([128, 4, CCH, 256], F16)
            wv_sb = wv_pool.tile([128, CCH, DIM], F16)
            wqk_sb = wqk_pool.tile([128, CCH, 2 * DIM], F16)
            wp_sb = wp_pool.tile([128, CCH, DIM], F16)

            bqk_sb = consts.tile([128, 12], F32)
            bvp_sb = consts.tile([1, 2 * DIM], F16)

            # warmup junk operand: memset'd, costs ~100ns on the DVE
            warm_sb = consts.tile([128, 512], F16)
            nc.vector.memset(warm_sb[:], 0.0)

            # input DMAs in consumption order; each is a plain strided copy
            # of a host-prepacked [128, c, free] block
            nc.sync.dma_start(x_sb[:, 0], xp_d[:, 0])
            nc.sync.dma_start(wv_sb[:, :, 0:512], wv_d[:, :, 0:512])
            nc.sync.dma_start(bvp_sb[:], bvp_d[:])
            nc.sync.dma_start(bqk_sb[:], bqk_d[:])
            nc.sync.dma_start(x_sb[:, 1], xp_d[:, 1])
            nc.sync.dma_start(wqk_sb[:, :, 0:256], wqk_d[:, :, 0:256])
            nc.sync.dma_start(x_sb[:, 2:4], xp_d[:, 2:4])
            nc.sync.dma_start(wqk_sb[:, :, 256:512], wqk_d[:, :, 256:512])
            nc.sync.dma_start(wv_sb[:, :, 512:DIM], wv_d[:, :, 512:DIM])
            nc.sync.dma_start(wqk_sb[:, :, 512:1024], wqk_d[:, :, 512:1024])
            nc.sync.dma_start(wqk_sb[:, :, 1024:1536], wqk_d[:, :, 1024:1536])
            nc.sync.dma_start(wp_sb[:], wp_d[:])

            # lift the HAM clock gate while the first DMAs land; junk MMs
            # have no input semaphores so they run right after the preamble
            for _ in range(8):
                ps = ps_pool.tile([128, 512], F32)
                nc.tensor.matmul(
                    ps[:], warm_sb[:, 0:128], warm_sb[:], start=True, stop=True,
                )

            # broadcast b_v across partitions once; the C evictions add it
            # on the DVE
            bvp32 = consts.tile([1, 2 * DIM], F32)
            nc.vector.tensor_copy(bvp32[:], bvp_sb[:])
            bvb = consts.tile([128, DIM], F32)
            nc.gpsimd.partition_broadcast(bvb[:], bvp32[:, 0:DIM], channels=128)

            qk_sb = qk_pool.tile([128, 12, N], F16)         # [o=1536, n]
            score_sb = score_pool.tile([128, CCH, N], F16)  # [c=768, n]
            v_sb = v_pool.tile([128, 8, N_HEAD * 65], F16)  # [n, h*(64+1)]

            # ---- Phase C: v projection, natural layout + ones cols ----
            v_ones = v_sb[:].rearrange("p n (h d) -> p n h d", d=65)[:, :, :, 64:65]
            nc.vector.memset(v_ones, 1.0)

            def c_group(nt, blk):
                o0, ow, off = ((0, 512, 0), (512, 256, 8 * 65))[blk]
                ps = ps_pool.tile([128, 512], F32)
                n0 = (nt % 2) * 128
                for c in range(CCH):
                    nc.tensor.matmul(
                        ps[:, :ow],
                        x_sb[:, nt // 2, c, n0:n0 + 128],
                        wv_sb[:, c, o0:o0 + ow],
                        start=(c == 0),
                        stop=(c == CCH - 1),
                    )
                nh = ow // 64
                src = ps[:, :ow].rearrange("p (h d) -> p h d", d=64)
                bias = bvb[:, o0:o0 + ow].rearrange("p (h d) -> p h d", d=64)
                dst = v_sb[:, nt, off:off + nh * 65].rearrange(
                    "p (h d) -> p h d", d=65
                )[:, :, 0:64]
                nc.vector.tensor_add(dst, src, bias)

            # blk-0 groups for nt 0..3 run upfront (they only need the first
            # x/wv descriptors); everything else becomes wave-0/1 fillers
            for nt in range(4):
                c_group(nt, 0)

            # ---- Phase B helper: one [o-tile, nq] strip of the qk^T proj ----
            def b_group(ot, nq):
                sl = SLOT[ot]
                ps = ps_pool.tile([128, 512], F32)
                g0 = nq // 256
                for c in range(CCH):
                    nc.tensor.matmul(
                        ps[:],
                        wqk_sb[:, c, sl * 128:(sl + 1) * 128],
                        x_sb[:, g0:g0 + 2, c, :],
                        start=(c == 0),
                        stop=(c == CCH - 1),
                    )
                nc.vector.tensor_scalar_add(
                    qk_sb[:, ot, nq:nq + 512], ps[:], bqk_sb[:, ot:ot + 1],
                )

            # ---- Phase D helpers ----
            def score_pair(hp, nq, nk):
                """scoresT for both heads of pair hp, one nk tile: head A into
                cols 0:512 (PE rows 0-63), head B into 512:1024 (rows 64-127),
                then exp straight from the 2-bank PSUM pair into fp16 SBUF."""
                pair = pair_pool.tile([128, 1024], F32)
                for half, p0 in ((0, 0), (1, 64)):
                    nc.tensor.matmul(
                        pair[:, half * 512:(half + 1) * 512],
                        qk_sb[p0:p0 + 64, 6 + hp, nk * 128:(nk + 1) * 128],
                        qk_sb[p0:p0 + 64, hp, nq:nq + 512],
                        start=True, stop=True,
                        tile_position=(p0, 0),
                    )
                at = attn_pool.tile([128, 1024], F16)
                nc.scalar.activation(at[:], pair[:], AF.Exp)
                return at

            def pv_group(hp, nq, half, p0, attns, pool=None):
                """attn @ [v|1] for one head/nq strip + normalize by rowsum."""
                h = 2 * hp + half
                acc = (pool or acc_pool).tile([65, 512], F32, name="pair" if pool else "acc")
                for nk in range(8):
                    nc.tensor.matmul(
                        acc[:],
                        v_sb[:, nk, h * 65:(h + 1) * 65],
                        attns[nk][:, half * 512:(half + 1) * 512],
                        start=(nk == 0),
                        stop=(nk == 7),
                    )
                # custom-DVE ops mis-read PSUM APs at partition offsets > 0 —
                # stage the rowsum row to SBUF first.
                rs = small_pool.tile([1, 512], F32, tag="rs")
                nc.vector.tensor_copy(rs[:], acc[64:65, :])
                rec = small_pool.tile([1, 512], F32, tag="rec")
                nc.vector.reciprocal_approx_fast(rec[:], rs[:])
                bc = small_pool.tile([64, 512], F32, tag="bc")
                nc.gpsimd.partition_broadcast(bc[:], rec[:], channels=64)
                nc.vector.tensor_mul(
                    score_sb[p0:p0 + 64, hp, nq:nq + 512], acc[0:64, :], bc[:],
                )

            # ---- Phases B + D interleaved in waves over head pairs ----
            # wave 0 starts as soon as b(0,0)/b(6,0) land: pairs that need
            # only the first x/wqk descriptors run first, so the ScalarE exp
            # stream (the wave-region bottleneck) starts ~7us earlier
            b_group(0, 0)
            b_group(6, 0)
            w0_pairs = (
                [(0, nk) for nk in range(4)]
                + [(512, nk) for nk in range(4)]
                + [(0, nk) for nk in range(4, 8)]
                + [(512, nk) for nk in range(4, 8)]
            )
            w0_fillers = [lambda: b_group(0, 512), lambda: b_group(6, 512)]
            for ot in (1, 7):
                for nq in (0, 512):
                    w0_fillers.append(lambda ot=ot, nq=nq: b_group(ot, nq))
            for nt in (0, 1, 2, 3):
                w0_fillers.append(lambda nt=nt: c_group(nt, 1))
            for nt in (4, 5, 6, 7):
                w0_fillers.append(lambda nt=nt: c_group(nt, 0))
            for nt in (4, 5, 6, 7):
                w0_fillers.append(lambda nt=nt: c_group(nt, 1))
            strips = {0: [None] * 8, 512: [None] * 8}
            fi = 0
            for si, (nq, nk) in enumerate(w0_pairs):
                strips[nq][nk] = score_pair(0, nq, nk)
                if fi < len(w0_fillers):
                    w0_fillers[fi]()
                    fi += 1
            while fi < len(w0_fillers):
                w0_fillers[fi]()
                fi += 1
            prev_strips = (0, strips)
            for hp in range(1, 5):
                # fillers keep the PE busy while ScalarE exps this wave
                fillers = []
                bfill = []
                if hp < 5:
                    for ot in (hp + 1, 7 + hp):
                        for nq in (0, 512):
                            bfill.append(lambda ot=ot, nq=nq: b_group(ot, nq))
                pfill = []
                if prev_strips is not None:
                    php, pstrips = prev_strips
                    for nq in (0, 512):
                        for half, p0 in ((0, 0), (1, 64)):
                            pfill.append(
                                lambda nq=nq, half=half, p0=p0, php=php,
                                       s=pstrips: pv_group(php, nq, half, p0, s[nq])
                            )
                # alternate B and PV fillers: spacing the PV groups apart
                # lets each normalize chain drain before its PSUM acc bank
                # is recycled
                for k in range(max(len(bfill), len(pfill))):
                    if k < len(bfill):
                        fillers.append(bfill[k])
                    if k < len(pfill):
                        fillers.append(pfill[k])
                strips = {0: [], 512: []}
                fi = 0
                for si, (nq, nk) in enumerate(
                    [(nq, nk) for nq in (0, 512) for nk in range(8)]
                ):
                    strips[nq].append(score_pair(hp, nq, nk))
                    if si % 2 == 1 and fi < len(fillers):
                        fillers[fi]()
                        fi += 1
                while fi < len(fillers):
                    fillers[fi]()
                    fi += 1
                prev_strips = (hp, strips)

            # ---- Phase E helper: ScalarE copy eviction (bias added on the
            # host), fp16 staging, DMA per o-half ----
            def e_tile(nt):
                stage = out_pool.tile([128, DIM], F16)
                for o0, ow in ((0, 512), (512, 256)):
                    ps = ps_pool.tile([128, 512], F32)
                    for c in range(CCH):
                        nc.tensor.matmul(
                            ps[:, :ow],
                            score_sb[:, c, nt * 128:(nt + 1) * 128],
                            wp_sb[:, c, o0:o0 + ow],
                            start=(c == 0),
                            stop=(c == CCH - 1),
                        )
                    nc.scalar.activation(
                        stage[:, o0:o0 + ow], ps[:, :ow], AF.Copy,
                    )
                    nc.sync.dma_start(
                        out_d[nt * 128:(nt + 1) * 128, o0:o0 + ow],
                        stage[:, o0:o0 + ow],
                    )

            # ---- wave 5: scores for hp=5; pv(4) fills the nq0 half, then
            # pv(5,nq0) fills the nq1 half; the tail interleaves pv(5,nq1)
            # with E tiles (whose nq halves of score_sb are already final) ----
            _, p4 = prev_strips
            strips5 = {0: [], 512: []}
            for nk in range(8):
                strips5[0].append(score_pair(5, 0, nk))
                if nk == 3:
                    pv_group(4, 0, 0, 0, p4[0])
                elif nk == 7:
                    pv_group(4, 0, 1, 64, p4[0])
            for nk in range(8):
                strips5[512].append(score_pair(5, 512, nk))
                if nk == 3:
                    pv_group(5, 0, 0, 0, strips5[0])
                elif nk == 7:
                    pv_group(5, 0, 1, 64, strips5[0])
            # pv4(nq1) waits until here: its strips are still alive and only
            # E tiles nt>=4 consume its output, so its chains hide under E
            pv_group(5, 512, 0, 0, strips5[512], pool=pair_pool)
            e_tile(0)
            pv_group(4, 512, 0, 0, p4[512])
            pv_group(5, 512, 1, 64, strips5[512], pool=pair_pool)
            e_tile(1)
            pv_group(4, 512, 1, 64, p4[512])
            for nt in (2, 3, 4, 5, 6, 7):
                e_tile(nt)

    nc.compile()
    return nc


def _get_nc():
    if "nc" not in _CACHE:
        _CACHE["nc"] = _build()
    return _CACHE["nc"]


def _pack6(w):
    """[768, o] -> [128, 6, o] with pack[p, c, o] = w[128c+p, o]."""
    return np.ascontiguousarray(w.reshape(CCH, 128, -1).transpose(1, 0, 2))


def kernel(x, w_qkv, b_qkv, w_proj, b_proj, **run_kwargs):
    x = np.asarray(x, dtype=np.float32)
    w_qkv = np.asarray(w_qkv, dtype=np.float32)
    b_qkv = np.asarray(b_qkv, dtype=np.float32)
    w_proj = np.asarray(w_proj, dtype=np.float32)
    b_proj = np.asarray(b_proj, dtype=np.float32)

    # Host-side layout prep (no arithmetic beyond folding the 1/sqrt(d) scale
    # into the q projection).
    w_qk = w_qkv[: 2 * DIM].copy()
    b_qk = b_qkv[: 2 * DIM].copy()
    w_qk[:DIM] *= SCALE
    b_qk[:DIM] *= SCALE
    # wqk in wave-order o-slots: pack[p, c, 128*slot+j] = w_qk[128*ot+j, 128c+p]
    wq4 = w_qk.reshape(12, 128, DIM)[OT_ORDER]            # [slot, j, c-dim]
    wqk_p = np.ascontiguousarray(
        wq4.transpose(2, 0, 1).reshape(DIM, 2 * DIM)      # [c-dim, slot*128+j]
    )
    wqk_p = _pack6(wqk_p).astype(np.float16)              # [128, 6, 1536]
    b_qk_t = np.ascontiguousarray(b_qk.reshape(12, 128).T)  # [128, 12] f32
    wv_p = _pack6(np.ascontiguousarray(w_qkv[2 * DIM:].T)).astype(np.float16)
    wp_p = _pack6(np.ascontiguousarray(w_proj.T)).astype(np.float16)
    b_vp = np.concatenate([b_qkv[2 * DIM:], b_proj]).reshape(1, -1).astype(np.float16)

    nc = _get_nc()
    in_maps = []
    for b in range(NB):
        xp = _pack6(np.ascontiguousarray(x[b].T)).astype(np.float16)
        # [128, 6, 1024] -> [128, 4 n-groups, 6, 256] for 3KB DMA runs
        xp = np.ascontiguousarray(
            xp.reshape(128, CCH, 4, 256).transpose(0, 2, 1, 3))
        in_maps.append({
            "xp": xp,
            "wqk_p": wqk_p,
            "b_qk_t": b_qk_t,
            "wv_p": wv_p,
            "wp_p": wp_p,
            "b_vp": b_vp,
        })
    res = run_bass_kernel_spmd(nc, in_maps, core_ids=list(range(NB)), **run_kwargs)
    out = np.stack(
        [res.results[b]["out"].astype(np.float32) for b in range(NB)], axis=0
    )
    out += b_proj
    if run_kwargs:
        return out, res
    return out


if __name__ == "__main__":
    rng = np.random.default_rng(0)
    x = rng.standard_normal((NB, N, DIM), dtype=np.float32)
    w_qkv = rng.standard_normal((3 * DIM, DIM), dtype=np.float32) * DIM ** -0.5
    b_qkv = rng.standard_normal((3 * DIM,), dtype=np.float32) * 0.02
    w_proj = rng.standard_normal((DIM, DIM), dtype=np.float32) * DIM ** -0.5
    b_proj = rng.standard_normal((DIM,), dtype=np.float32) * 0.02
    out = kernel(x=x, w_qkv=w_qkv, b_qkv=b_qkv, w_proj=w_proj, b_proj=b_proj)
    print("out", out.shape, out.dtype, float(np.abs(out).mean()))
